# revision 22
# baseline (speedup 1.0000x reference)
"""Trainium2 Bass kernel for nn_AnonTokyoEncoder (sparse_attention).

Sharding: 8 cores, half-scene per core (core c: scene c//2, token-half c%2).
Map queries 256/core, agent queries 64/core; KV token sets are kept full via
two pair-AllGathers per layer (map halves after mm, agent halves after am).

Attention: dense masked form of the exact top-32 attention. -d2 is computed
on device in fp32, the 32nd-largest threshold per query comes from 4 rounds
of max8+match_replace, and a 0/1 mask multiplies the exp'd scores, which is
mathematically identical to gathering the top-32 keys (softmax is permutation
invariant and 0-weight entries drop out). Compute dtype bf16, fp32 PSUM.
"""
import os
import sys

sys.path.insert(0, "/opt/trn_rl_repo")

import numpy as np
import ml_dtypes

import concourse.bass as bass
import concourse.mybir as mybir
from concourse import bacc, tile
from concourse.bass_utils import run_bass_kernel_spmd

dt = mybir.dt
F32, BF16 = dt.float32, dt.bfloat16
ALU = mybir.AluOpType
AF = mybir.ActivationFunctionType
AX = mybir.AxisListType

B, A, T, M, P = 4, 128, 11, 512, 20
D, H, K, L = 256, 8, 32, 6
HD = D // H
NEG = -1e9
MQ = M // 2          # map queries per core (256)
AQ = A // 2          # agent queries per core (64)
AT = AQ * T          # agent pointnet tokens per core (704)
MT = MQ * P          # map pointnet tokens per core (5120)

bf16 = ml_dtypes.bfloat16
LAST_EXEC_TIME_NS = None


# ---------------------------------------------------------------- host prep
def _np(x):
    return np.asarray(x, dtype=np.float32)


def _pack_w(w):
    """[din, dout] -> [128, nchin*dout], chunk-major on din. din<=128 passthrough."""
    w = _np(w)
    din, dout = w.shape
    if din <= 128:
        return w
    nch = din // 128
    return w.reshape(nch, 128, dout).transpose(1, 0, 2).reshape(128, nch * dout)


def _augment_agent(obj_trajs, obj_mask_b, obj_trajs_mask):
    b, a, t, _ = obj_trajs.shape
    pos_size = obj_trajs[..., 0:6]
    type_oh = np.zeros((b, a, t, 5), np.float32)
    eye = np.broadcast_to(np.eye(t, dtype=np.float32), (b, a, t, t))
    ts = np.broadcast_to(
        np.linspace(0.0, 1.0, t, dtype=np.float32)[:, None], (b, a, t, 1)
    )
    heading = obj_trajs[..., 6:8]
    vel = obj_trajs[..., 8:10]
    vel_pre = np.roll(vel, 1, axis=2)
    acce = (vel - vel_pre) / 0.1
    acce[:, :, 0] = acce[:, :, 1]
    out = np.concatenate(
        [pos_size, type_oh, eye, ts, heading, vel, acce], -1
    )  # 29
    out = np.where(obj_mask_b[..., None], out, 0.0)
    return np.concatenate([out, obj_trajs_mask[..., None]], -1)  # 30


def _augment_map(map_polys, poly_mask_b):
    xy = map_polys[..., 0:2]
    pre = np.roll(xy, 1, axis=2)
    pre[:, :, 0] = pre[:, :, 1]
    out = np.concatenate([map_polys, pre], -1)  # 9
    return np.where(poly_mask_b[..., None], out, 0.0)


def _qpos_feats(pos):
    # rows: 2x, 2y, -(x^2+y^2), 1   (paired with key rows x, y, 1, -(x^2+y^2))
    x, y = pos[:, 0], pos[:, 1]
    return np.stack([2 * x, 2 * y, -(x * x + y * y), np.ones_like(x)], 0)


def _kpos_feats(pos, mask_b):
    x, y = pos[:, 0], pos[:, 1]
    k2 = x * x + y * y + np.where(mask_b, 0.0, 1e9)
    return np.stack([x, y, np.ones_like(x), -k2], 0)


def prepare_inputs(inputs):
    """Build the per-core in_maps (8 dicts of numpy arrays)."""
    p = inputs["params"]
    obj_trajs = _np(inputs["obj_trajs"])
    obj_trajs_mask = _np(inputs["obj_trajs_mask"])
    obj_positions = _np(inputs["obj_positions"])
    agent_mask = _np(inputs["agent_mask"])
    map_polylines = _np(inputs["map_polylines"])
    map_polylines_mask = _np(inputs["map_polylines_mask"])
    map_centers = _np(inputs["map_polylines_center"])
    map_mask = _np(inputs["map_mask"])

    agent_in = _augment_agent(obj_trajs, obj_trajs_mask > 0.5, obj_trajs_mask)
    map_in = _augment_map(map_polylines, map_polylines_mask > 0.5)

    # ---- weights (shared across cores)
    wmap = {}

    def put(name, arr, dtype=bf16):
        wmap[name] = np.ascontiguousarray(np.asarray(arr, np.float32)).astype(dtype)

    ae, me = p["agent_enc"], p["map_enc"]
    put("aw_pre", _pack_w(ae["pre"][0][0]))
    put("ab_pre", _np(ae["pre"][0][1]).reshape(-1,128).T, np.float32)
    put("aw_post1", _pack_w(ae["post"][0][0]))
    put("ab_post1", _np(ae["post"][0][1]).reshape(-1,128).T, np.float32)
    put("aw_post2", _pack_w(ae["post"][1][0]))
    put("ab_post2", _np(ae["post"][1][1]).reshape(-1,128).T, np.float32)
    put("aw_out", _pack_w(ae["out"][0]))
    put("ab_out", _np(ae["out"][1]).reshape(-1,128).T, np.float32)
    for i in range(3):
        put(f"mw_pre{i}", _pack_w(me["pre"][i][0]))
        put(f"mb_pre{i}", _np(me["pre"][i][1]).reshape(-1,1), np.float32)
    put("mw_post1", _pack_w(me["post"][0][0]))
    put("mb_post1", _np(me["post"][0][1]).reshape(-1,1), np.float32)
    put("mw_post2", _pack_w(me["post"][1][0]))
    put("mb_post2", _np(me["post"][1][1]).reshape(-1,1), np.float32)
    put("mw_out", _pack_w(me["out"][0]))
    put("mb_out", _np(me["out"][1]).reshape(-1,128).T, np.float32)

    for l, lp in enumerate(p["layers"]):
        for blk in ("mm", "aa", "am"):
            bp = lp[blk]
            wq = _np(bp["q"][0]) / np.sqrt(HD)
            bq = _np(bp["q"][1]) / np.sqrt(HD)
            parts = [
                _pack_w(wq), _pack_w(bp["k"][0]), _pack_w(bp["v"][0]),
                _pack_w(bp["o"][0]), _pack_w(bp["ffn1"][0]), _pack_w(bp["ffn2"][0]),
            ]
            put(f"L{l}{blk}_W", np.concatenate(parts, axis=1))
            bo = _np(bp["o"][1]) + _np(bp["o"][0]).T @ _np(bp["v"][1])
            bias = np.concatenate(
                [bq, _np(bp["k"][1]), np.zeros(256, np.float32), bo,
                 _np(bp["ffn1"][1]), _np(bp["ffn2"][1])]
            )
            put(f"L{l}{blk}_b", bias.reshape(-1, 128).T, np.float32)
            g1, b1 = bp["norm"]
            g2, b2 = bp["ffn_norm"]
            ln = np.stack([np.concatenate([_np(g1), _np(g2)]),
                           np.concatenate([_np(b1), _np(b2)])], 0)
            put(f"L{l}{blk}_ln", ln, np.float32)

    wmap["ident"] = np.eye(128, dtype=np.float32)
    rep = np.zeros((128, 128), np.float32)
    for r in range(4):
        rep[32 * r, 32 * r:32 * r + 32] = 1.0
    wmap["repmat"] = rep.astype(bf16)

    in_maps = []
    for c in range(8):
        s, h = c // 2, c % 2
        im = dict(wmap)
        ai = agent_in[s, h * AQ:(h + 1) * AQ]            # [64, 11, 30]
        im["agent_in"] = ai.transpose(2, 0, 1).reshape(30, AT).astype(bf16)
        mi = map_in[s, h * MQ:(h + 1) * MQ]              # [256, 20, 9]
        im["map_in"] = mi.transpose(2, 0, 1).reshape(9, MT).astype(bf16)
        im["mm_qpos"] = _qpos_feats(map_centers[s, h * MQ:(h + 1) * MQ]).astype(np.float32)
        im["mm_kpos"] = _kpos_feats(map_centers[s], map_mask[s] > 0.5).astype(np.float32)
        im["aa_qpos"] = _qpos_feats(obj_positions[s, h * AQ:(h + 1) * AQ]).astype(np.float32)
        im["aa_kpos"] = _kpos_feats(obj_positions[s], agent_mask[s] > 0.5).astype(np.float32)
        in_maps.append(im)
    return in_maps


# ------------------------------------------------------------- device build
def chunks(d):
    return (d + 127) // 128


def build(nlayers=L):
    nc = bacc.Bacc("TRN2", target_bir_lowering=False, debug=False, num_devices=8)

    params = {}

    def par(name, shape, dtype):
        params[name] = nc.dram_tensor(name, list(shape), dtype, kind="ExternalInput").ap()
        return params[name]

    # encoder weights
    par("aw_pre", (30, 256), BF16); par("ab_pre", (128, 2), F32)
    par("aw_post1", (128, 4 * 256), BF16); par("ab_post1", (128, 2), F32)
    par("aw_post2", (128, 2 * 256), BF16); par("ab_post2", (128, 2), F32)
    par("aw_out", (128, 2 * 256), BF16); par("ab_out", (128, 2), F32)
    par("mw_pre0", (9, 64), BF16); par("mb_pre0", (64, 1), F32)
    par("mw_pre1", (64, 64), BF16); par("mb_pre1", (64, 1), F32)
    par("mw_pre2", (64, 64), BF16); par("mb_pre2", (64, 1), F32)
    par("mw_post1", (128, 64), BF16); par("mb_post1", (64, 1), F32)
    par("mw_post2", (64, 64), BF16); par("mb_post2", (64, 1), F32)
    par("mw_out", (64, 256), BF16); par("mb_out", (128, 2), F32)
    for l in range(nlayers):
        for blk in ("mm", "aa", "am"):
            par(f"L{l}{blk}_W", (128, 6144), BF16)
            par(f"L{l}{blk}_b", (128, 18), F32)
            par(f"L{l}{blk}_ln", (2, 512), F32)
    par("ident", (128, 128), F32)
    par("repmat", (128, 128), BF16)
    par("agent_in", (30, AT), BF16)
    par("map_in", (9, MT), BF16)
    par("mm_qpos", (4, MQ), F32)
    par("mm_kpos", (4, M), F32)
    par("aa_qpos", (4, AQ), F32)
    par("aa_kpos", (4, A), F32)

    out_map = nc.dram_tensor("out_map", [128, 2 * MQ], F32, kind="ExternalOutput").ap()
    out_agent = nc.dram_tensor("out_agent", [128, 2 * AQ], F32, kind="ExternalOutput").ap()

    import contextlib
    with tile.TileContext(nc) as tc, contextlib.ExitStack() as stack:
        sb = stack.enter_context(tc.tile_pool(name="sb", bufs=1))
        sc = stack.enter_context(tc.tile_pool(name="sc", bufs=2))
        me = stack.enter_context(tc.tile_pool(name="me", bufs=2))
        one = stack.enter_context(tc.tile_pool(name="one", bufs=1))
        wp = stack.enter_context(tc.tile_pool(name="wp", bufs=2))
        un = stack.enter_context(tc.tile_pool(name="un", bufs=8, space="PSUM"))
        dr = stack.enter_context(tc.tile_pool(name="dr", bufs=4, space="DRAM"))

        def dma(dst, src):
            nc.sync.dma_start(out=dst, in_=src)

        # ---- persistent tiles
        ones = sb.tile([128, 512], BF16, tag="ones")
        nc.vector.memset(ones[:, :], 1.0)
        onesf = sb.tile([128, 1], F32, tag="onesf")
        nc.vector.memset(onesf[:, :], 1.0)
        onesfr = sb.tile([1, 128], F32, tag="onesfr")
        nc.vector.memset(onesfr[:, :], 1.0)
        epsb = sb.tile([128, 1], F32, tag="epsb")
        nc.vector.memset(epsb[:, :], 1e-5)
        ident = sb.tile([128, 128], F32, tag="ident")
        dma(ident[:, :], params["ident"][:, :])
        repmat = sb.tile([128, 128], BF16, tag="repmat")
        dma(repmat[:, :], params["repmat"][:, :])

        def load(name, shape, dtype, pool=sb, tag=None):
            t = pool.tile(list(shape), dtype, tag=tag or name)
            dma(t[:, :], params[name][:, :])
            return t

        # ============================================================
        # helper: generic feature-major linear  out[dout, ntok] (bf16)
        #   X: tile [128, nchin*ntok] (chunk-major) or list of (tile, col0, ntok_src)
        # ============================================================
        def linear_fm(Xc, din, dout, ntok, Wt, wof, bt, bof, act=AF.Copy,
                      out_t=None, res=None, out_dt=BF16, pool=sc, tag="lin"):
            """Xc(ji, s0, s1) -> AP of input chunk ji, token slice [s0:s1].
            res: optional residual tile (same layout as out) added via DVE.
            Returns out tile [dsz, nchout*ntok]."""
            nci, nco = chunks(din), chunks(dout)
            dsz = min(din, 128)
            osz = min(dout, 128)
            if out_t is None:
                out_t = pool.tile([osz, nco * ntok], out_dt, tag=tag)
            for jo in range(nco):
                od = min(128, dout - jo * 128)
                for s0 in range(0, ntok, 512):
                    s1 = min(ntok, s0 + 512)
                    ps = un.tile([osz, 512], F32, tag="u")
                    for ji in range(nci):
                        idd = min(128, din - ji * 128)
                        nc.tensor.matmul(
                            ps[0:od, 0:s1 - s0],
                            Wt[0:idd, wof + ji * dout + jo * 128: wof + ji * dout + jo * 128 + od],
                            Xc(ji, s0, s1),
                            start=(ji == 0), stop=(ji == nci - 1))
                    dst = out_t[0:od, jo * ntok + s0: jo * ntok + s1]
                    if res is not None:
                        nc.vector.scalar_tensor_tensor(
                            dst, ps[0:od, 0:s1 - s0], bt[0:od, bof + jo: bof + jo + 1],
                            res[0:od, jo * ntok + s0: jo * ntok + s1], ALU.add, ALU.add)
                    else:
                        fn = AF.Identity if act == AF.Copy else act
                        nc.scalar.activation(dst, ps[0:od, 0:s1 - s0], fn,
                                             bias=bt[0:od, bof + jo: bof + jo + 1])
            return out_t

        def xch(Xt, ntok):
            return lambda ji, s0, s1: Xt[:, ji * ntok + s0: ji * ntok + s1]

        # token-major linear (for V): out [128, ntokchunks*dout]
        def linear_tok(Xt, din, dout, ntok, Wt, wof, bt, bof, pool=sc, tag="ltok"):
            nci = chunks(din)
            ntc = ntok // 128
            out_t = pool.tile([128, ntc * dout], BF16, tag=tag)
            for to in range(ntc):
                ps = un.tile([128, 512], F32, tag="u")
                for ji in range(nci):
                    nc.tensor.matmul(
                        ps[:, 0:dout],
                        Xt[:, ji * ntok + to * 128: ji * ntok + (to * 128 + 128)],
                        Wt[:, wof + ji * dout: wof + (ji + 1) * dout],
                        start=(ji == 0), stop=(ji == nci - 1))
                nc.scalar.activation(
                    out_t[:, to * dout:(to + 1) * dout], ps[:, 0:dout], AF.Copy)
            return out_t

        # layernorm on feature-major [256, ntok] bf16, with gamma/beta f32 [2,512] at col gof
        def layer_norm(Xt, ntok, lnt, gof, pool=sc, tag="lnout"):
            """Xt: f32 [128, 2*ntok]. Returns (bf16 tile, f32 tile)."""
            ntc = (ntok + 127) // 128
            x2 = sc.tile([128, 2 * ntok], F32, tag="x2")
            nc.vector.tensor_tensor(x2[:, :2 * ntok], Xt[:, :2 * ntok], Xt[:, :2 * ntok], ALU.mult)
            rows_d = sc.tile([1, 512], F32, tag="rowsd")
            rows_c = sc.tile([2, 512], F32, tag="rowsc")
            for tqc in range(ntc):
                tn = min(128, ntok - tqc * 128)
                st = un.tile([128, 2], F32, tag="u")
                for ji in range(2):
                    nc.tensor.matmul(st[0:tn, 0:1],
                                     Xt[:, ji * ntok + tqc * 128: ji * ntok + tqc * 128 + tn],
                                     onesf[0:128, 0:1],
                                     start=(ji == 0), stop=(ji == 1))
                for ji in range(2):
                    nc.tensor.matmul(st[0:tn, 1:2],
                                     x2[:, ji * ntok + tqc * 128: ji * ntok + tqc * 128 + tn],
                                     onesf[0:128, 0:1],
                                     start=(ji == 0), stop=(ji == 1))
                tr = sc.tile([128, 3], F32, tag="tr")
                mu = sc.tile([128, 1], F32, tag="mu")
                nc.vector.tensor_scalar_mul(mu[0:tn, :], st[0:tn, 0:1], 1.0 / D)
                var = sc.tile([128, 1], F32, tag="var")
                nc.vector.tensor_scalar_mul(var[0:tn, :], st[0:tn, 1:2], 1.0 / D)
                # var = m2 - mu^2 computed as -(mu*mu - m2)
                nc.vector.scalar_tensor_tensor(var[0:tn, :], mu[0:tn, :], mu[0:tn, :],
                                               var[0:tn, :], ALU.mult, ALU.subtract)
                nc.vector.tensor_scalar_mul(var[0:tn, :], var[0:tn, :], -1.0)
                nc.vector.tensor_scalar_add(var[0:tn, :], var[0:tn, :], 1e-5)
                lnv = sc.tile([128, 1], F32, tag="lnv")
                nc.scalar.activation(lnv[0:tn, :], var[0:tn, :], AF.Ln)
                r0 = sc.tile([128, 1], F32, tag="r0")
                nc.scalar.activation(r0[0:tn, :], lnv[0:tn, :], AF.Exp, scale=-0.5)
                # Newton: r = r0*(1.5 - 0.5*v*r0^2)
                t1 = sc.tile([128, 1], F32, tag="t1")
                nc.vector.tensor_tensor(t1[0:tn, :], r0[0:tn, :], r0[0:tn, :], ALU.mult)
                nc.vector.scalar_tensor_tensor(t1[0:tn, :], var[0:tn, :], -0.5,
                                               t1[0:tn, :], ALU.mult, ALU.mult)
                nc.vector.tensor_scalar_add(t1[0:tn, :], t1[0:tn, :], 1.5)
                nc.vector.tensor_tensor(tr[0:tn, 0:1], r0[0:tn, :], t1[0:tn, :], ALU.mult)
                nc.vector.scalar_tensor_tensor(tr[0:tn, 1:2], mu[0:tn, :], -1.0,
                                               tr[0:tn, 0:1], ALU.mult, ALU.mult)
                nc.vector.memset(tr[0:tn, 2:3], 1.0)
                rp = un.tile([128, 512], F32, tag="u")
                nc.tensor.matmul(rp[0:1, 0:tn], tr[0:tn, 0:1], ident[0:tn, 0:tn],
                                 start=True, stop=True)
                nc.tensor.matmul(rp[0:2, 256:256 + tn], tr[0:tn, 1:3], ident[0:tn, 0:tn],
                                 start=True, stop=True)
                nc.scalar.activation(rows_d[0:1, tqc * 128: tqc * 128 + tn],
                                     rp[0:1, 0:tn], AF.Copy)
                nc.scalar.activation(rows_c[0:2, tqc * 128: tqc * 128 + tn],
                                     rp[0:2, 256:256 + tn], AF.Copy)
            out_f = pool.tile([128, 2 * ntok], F32, tag=tag + "f")
            out_b = pool.tile([128, 2 * ntok], BF16, tag=tag)
            for jo in range(2):
                dps = un.tile([128, 512], F32, tag="u")
                nc.tensor.matmul(dps[:, 0:ntok], lnt[0:1, gof + jo * 128: gof + jo * 128 + 128],
                                 rows_d[0:1, 0:ntok], start=True, stop=True)
                cps = un.tile([128, 512], F32, tag="u")
                nc.tensor.matmul(cps[:, 0:ntok], lnt[0:2, gof + jo * 128: gof + jo * 128 + 128],
                                 rows_c[0:2, 0:ntok], start=True, stop=True)
                tmp = sc.tile([128, 2 * 512], F32, tag="lntmp")
                nc.vector.tensor_tensor(tmp[:, 0:ntok], Xt[:, jo * ntok:(jo + 1) * ntok],
                                        dps[:, 0:ntok], ALU.mult)
                nc.vector.tensor_tensor(out_f[:, jo * ntok:(jo + 1) * ntok],
                                        tmp[:, 0:ntok], cps[:, 0:ntok], ALU.add)
                nc.vector.tensor_copy(out_b[:, jo * ntok:(jo + 1) * ntok],
                                      out_f[:, jo * ntok:(jo + 1) * ntok])
            return out_b, out_f

        # masked topk threshold -> m01 [128, ntkc*ntq] bf16
        def build_mask(qp_name, kp_name, ntq, ntk, mtag):
            qp = load(qp_name, (4, ntq), F32, pool=sc, tag="qp")
            kp = load(kp_name, (4, ntk), F32, pool=sc, tag="kp")
            ntqc, ntkc = (ntq + 127) // 128, ntk // 128
            t32 = sc.tile([128, ntqc], F32, tag="t32")
            for tqc in range(ntqc):
                tn = min(128, ntq - tqc * 128)
                ps = un.tile([128, 512], F32, tag="u")
                nc.tensor.matmul(ps[0:tn, 0:ntk], qp[0:4, tqc * 128: tqc * 128 + tn],
                                 kp[0:4, 0:ntk], start=True, stop=True)
                vals = sc.tile([128, 512], F32, tag="mvals")
                nc.scalar.activation(vals[0:tn, 0:ntk], ps[0:tn, 0:ntk], AF.Copy)
                m8 = sc.tile([128, 8], F32, tag="m8")
                for r in range(4):
                    nc.vector.max(m8[0:tn, :], vals[0:tn, 0:ntk])
                    if r < 3:
                        nc.vector.match_replace(vals[0:tn, 0:ntk], m8[0:tn, :],
                                                vals[0:tn, 0:ntk], -1e30)
                nc.vector.tensor_copy(t32[0:tn, tqc:tqc + 1], m8[0:tn, 7:8])
            # transpose t32 -> row, replicate
            t32r_sb = sc.tile([1, 512], F32, tag="t32r")
            for tqc in range(ntqc):
                tn = min(128, ntq - tqc * 128)
                rp = un.tile([128, 512], F32, tag="u")
                nc.tensor.matmul(rp[0:1, 0:tn], t32[0:tn, tqc:tqc + 1],
                                 ident[0:tn, 0:tn], start=True, stop=True)
                nc.scalar.activation(t32r_sb[0:1, tqc * 128: tqc * 128 + tn],
                                     rp[0:1, 0:tn], AF.Copy)
            t32rep = sc.tile([128, 512], F32, tag="t32rep")
            rp2 = un.tile([128, 512], F32, tag="u")
            nc.tensor.matmul(rp2[:, 0:ntq], onesfr[0:1, 0:128],
                             t32r_sb[0:1, 0:ntq], start=True, stop=True)
            nc.scalar.activation(t32rep[:, 0:ntq], rp2[:, 0:ntq], AF.Copy)
            m01 = sb.tile([128, ntkc * ntq], BF16, tag=mtag)
            for c in range(ntkc):
                ps = un.tile([128, 512], F32, tag="u")
                nc.tensor.matmul(ps[:, 0:ntq], kp[0:4, c * 128:(c + 1) * 128],
                                 qp[0:4, 0:ntq], start=True, stop=True)
                nc.vector.tensor_tensor(m01[:, c * ntq:(c + 1) * ntq],
                                        ps[:, 0:ntq], t32rep[:, 0:ntq], ALU.is_ge)
            return m01

        # dense masked attention. q_fm [128,2*ntq], k_fm [128,2*ntk], v_tok [128,(ntk//128)*256]
        def attention(q_fm, k_fm, v_tok, m01, ntq, ntk, pool=sc, tag="attn"):
            ntkc = ntk // 128
            attn = pool.tile([128, 2 * ntq], BF16, tag=tag)
            if os.environ.get("KB_NOATT"):
                nc.vector.tensor_copy(attn[:, 0:2 * ntq], q_fm[:, 0:2 * ntq])
                return attn
            for g in range(2):
                den = un.tile([128, 512], F32, tag="u")
                av = un.tile([128, 512], F32, tag="u")

                def issue_avden(em_t, c):
                    for j in range(4):
                        nc.tensor.matmul(
                            av[32 * j:32 * j + 32, 0:ntq],
                            v_tok[:, c * 256 + 32 * (4 * g + j): c * 256 + 32 * (4 * g + j) + 32],
                            em_t[:, j * 512: j * 512 + ntq],
                            start=(c == 0), stop=(c == ntkc - 1),
                            tile_position=(0, 32 * j))
                        nc.tensor.matmul(
                            den[32 * j:32 * j + 32, 0:ntq],
                            ones[0:128, 0:32],
                            em_t[:, j * 512: j * 512 + ntq],
                            start=(c == 0), stop=(c == ntkc - 1),
                            tile_position=(0, 32 * j))

                pending = None
                for c in range(ntkc):
                    em = sc.tile([128, 4 * 512], BF16, tag="em")
                    sps_l = []
                    for j in range(4):
                        sps = un.tile([128, 512], F32, tag="u")
                        nc.tensor.matmul(
                            sps[:, 0:ntq],
                            k_fm[32 * j:32 * j + 32, g * ntk + c * 128: g * ntk + (c + 1) * 128],
                            q_fm[32 * j:32 * j + 32, g * ntq:(g + 1) * ntq],
                            start=True, stop=True, tile_position=(32 * j, 0))
                        sps_l.append(sps)
                    for j in range(4):
                        nc.scalar.activation(em[:, j * 512: j * 512 + ntq],
                                             sps_l[j][:, 0:ntq], AF.Exp)
                    for j in range(4):
                        nc.vector.tensor_tensor(em[:, j * 512: j * 512 + ntq],
                                                em[:, j * 512: j * 512 + ntq],
                                                m01[:, c * ntq:(c + 1) * ntq], ALU.mult)
                    if pending is not None:
                        issue_avden(*pending)
                    pending = (em, c)
                issue_avden(*pending)
                rrep = sc.tile([128, 512], F32, tag="rrep")
                nc.vector.reciprocal(rrep[:, 0:ntq], den[:, 0:ntq])
                nc.vector.tensor_tensor(attn[:, g * ntq:(g + 1) * ntq],
                                        av[:, 0:ntq], rrep[:, 0:ntq], ALU.mult)
            return attn

        # transformer block
        def block(Wt, bt, lnt, x_q, x_qf, ntq, x_kv, ntk, m01, out_tag):
            """x_q bf16 / x_qf f32 same content. Returns (bf16, f32) pair."""
            OQ, OK_, OV, OO = 0, 512, 1024, 1536
            OF1, OF2 = 2048, 4096
            BQ, BK, BV, BO, BF1, BF2 = 0, 2, 4, 6, 8, 16
            q_fm = linear_fm(xch(x_q, ntq), D, D, ntq, Wt, OQ, bt, BQ, tag="qfm")
            k_fm = linear_fm(xch(x_kv, ntk), D, D, ntk, Wt, OK_, bt, BK, tag="kfm")
            v_tok = linear_tok(x_kv, D, D, ntk, Wt, OV, bt, BV, tag="vtok")
            at = attention(q_fm, k_fm, v_tok, m01, ntq, ntk)
            x1 = linear_fm(xch(at, ntq), D, D, ntq, Wt, OO, bt, BO, res=x_qf,
                           out_dt=F32, tag="x1")
            x1n, x1nf = layer_norm(x1, ntq, lnt, 0, tag="x1n")
            hh = linear_fm(xch(x1n, ntq), D, 4 * D, ntq, Wt, OF1, bt, BF1,
                           act=AF.Relu, tag="ffnh")
            x2_ = linear_fm(xch(hh, ntq), 4 * D, D, ntq, Wt, OF2, bt, BF2,
                            res=x1nf, out_dt=F32, tag="x2r")
            return layer_norm(x2_, ntq, lnt, 256, pool=sb, tag=out_tag)

        # pair AllGather: my half [128, 2*ntok] bf16 -> full [128, 2*(2*ntok)]
        def allgather(x_half, ntok, tag):
            ib = dr.tile([128, 2 * ntok], BF16, tag="agin")
            nc.gpsimd.dma_start(ib[:, :], x_half[:, 0:2 * ntok])
            ob = dr.tile([2 * 128, 2 * ntok], BF16, tag="agout")
            nc.gpsimd.collective_compute(
                "AllGather", ALU.bypass,
                replica_groups=[[0, 1], [2, 3], [4, 5], [6, 7]],
                ins=[ib.opt()], outs=[ob.opt()])
            full = sb.tile([128, 2 * (2 * ntok)], BF16, tag=tag)
            for r in range(2):
                for ji in range(2):
                    dma(full[:, ji * 2 * ntok + r * ntok: ji * 2 * ntok + (r + 1) * ntok],
                        ob[r * 128:(r + 1) * 128, ji * ntok:(ji + 1) * ntok])
            return full

        # ================= encoders =================
        # agent pointnet
        a_in = load("agent_in", (30, AT), BF16, pool=one, tag="a_in")
        aw_pre = load("aw_pre", (30, 256), BF16, pool=sc, tag="wenc")
        ab_pre = load("ab_pre", (128, 2), F32, pool=sc, tag="benc")
        h1 = linear_fm(lambda ji, s0, s1: a_in[0:30, s0:s1], 30, 256, AT,
                       aw_pre, 0, ab_pre, 0, act=AF.Relu, pool=me, tag="ae")
        pooled = one.tile([128, 2 * AQ], BF16, tag="apool")
        for ji in range(2):
            nc.vector.tensor_reduce(
                pooled[:, ji * AQ:(ji + 1) * AQ],
                h1[:, ji * AT:(ji + 1) * AT].rearrange("p (a t) -> p a t", t=T),
                AX.X, ALU.max)
        prep = one.tile([128, 2 * AT], BF16, tag="aprep")
        for ji in range(2):
            nc.vector.tensor_copy(
                prep[:, ji * AT:(ji + 1) * AT].rearrange("p (a t) -> p a t", t=T),
                pooled[:, ji * AQ:(ji + 1) * AQ]
                .rearrange("p (a o) -> p a o", o=1).broadcast_to([128, AQ, T]))
        aw_p1 = load("aw_post1", (128, 4 * 256), BF16, pool=sc, tag="wenc")
        ab_p1 = load("ab_post1", (128, 2), F32, pool=sc, tag="benc")

        def cat_h1_prep(ji, s0, s1):
            if ji < 2:
                return h1[:, ji * AT + s0: ji * AT + s1]
            return prep[:, (ji - 2) * AT + s0: (ji - 2) * AT + s1]

        h2 = linear_fm(cat_h1_prep, 512, 256, AT, aw_p1, 0, ab_p1, 0,
                       act=AF.Relu, pool=me, tag="ae")
        aw_p2 = load("aw_post2", (128, 2 * 256), BF16, pool=sc, tag="wenc")
        ab_p2 = load("ab_post2", (128, 2), F32, pool=sc, tag="benc")
        h3 = linear_fm(xch(h2, AT), 256, 256, AT, aw_p2, 0, ab_p2, 0,
                       act=AF.Relu, pool=me, tag="ae")
        afeat = one.tile([128, 2 * AQ], BF16, tag="afeat")
        for ji in range(2):
            nc.vector.tensor_reduce(
                afeat[:, ji * AQ:(ji + 1) * AQ],
                h3[:, ji * AT:(ji + 1) * AT].rearrange("p (a t) -> p a t", t=T),
                AX.X, ALU.max)
        aw_o = load("aw_out", (128, 2 * 256), BF16, pool=sc, tag="wenc")
        ab_o = load("ab_out", (128, 2), F32, pool=sc, tag="benc")
        agent_mine = linear_fm(xch(afeat, AQ), 256, 256, AQ, aw_o, 0, ab_o, 0,
                               pool=sb, tag="agent_mine")
        agent_mine_f = sb.tile([128, 2 * AQ], F32, tag="agent_mine_f")
        nc.vector.tensor_copy(agent_mine_f[:, :], agent_mine[:, :])

        # map pointnet
        m_in = load("map_in", (9, MT), BF16, pool=one, tag="m_in")
        mw0 = load("mw_pre0", (9, 64), BF16, pool=sc, tag="wenc")
        mb0 = load("mb_pre0", (64, 1), F32, pool=sc, tag="benc")
        g1 = linear_fm(lambda ji, s0, s1: m_in[0:9, s0:s1], 9, 64, MT,
                       mw0, 0, mb0, 0, act=AF.Relu, pool=me, tag="me")
        mw1 = load("mw_pre1", (64, 64), BF16, pool=sc, tag="wenc")
        mb1 = load("mb_pre1", (64, 1), F32, pool=sc, tag="benc")
        g2 = linear_fm(xch(g1, MT), 64, 64, MT, mw1, 0, mb1, 0, act=AF.Relu, pool=me, tag="me")
        mw2 = load("mw_pre2", (64, 64), BF16, pool=sc, tag="wenc")
        mb2 = load("mb_pre2", (64, 1), F32, pool=sc, tag="benc")
        g3 = linear_fm(xch(g2, MT), 64, 64, MT, mw2, 0, mb2, 0, act=AF.Relu, pool=me, tag="me")
        mcat = me.tile([128, MT], BF16, tag="me")
        nc.vector.tensor_copy(mcat[0:64, :], g3[0:64, :])
        mpool = one.tile([64, MQ], BF16, tag="mpool")
        nc.vector.tensor_reduce(
            mpool[0:64, :], g3[0:64, :].rearrange("p (a t) -> p a t", t=P),
            AX.X, ALU.max)
        nc.vector.tensor_copy(
            mcat[64:128, :].rearrange("p (a t) -> p a t", t=P),
            mpool[0:64, :].rearrange("p (a o) -> p a o", o=1).broadcast_to([64, MQ, P]))
        mw_p1 = load("mw_post1", (128, 64), BF16, pool=sc, tag="wenc")
        mb_p1 = load("mb_post1", (64, 1), F32, pool=sc, tag="benc")
        g4 = linear_fm(xch(mcat, MT), 128, 64, MT, mw_p1, 0, mb_p1, 0,
                       act=AF.Relu, pool=me, tag="me")
        mw_p2 = load("mw_post2", (64, 64), BF16, pool=sc, tag="wenc")
        mb_p2 = load("mb_post2", (64, 1), F32, pool=sc, tag="benc")
        g5 = linear_fm(xch(g4, MT), 64, 64, MT, mw_p2, 0, mb_p2, 0,
                       act=AF.Relu, pool=me, tag="me")
        mfeat = one.tile([64, MQ], BF16, tag="mfeat")
        nc.vector.tensor_reduce(
            mfeat[0:64, :], g5[0:64, :].rearrange("p (a t) -> p a t", t=P),
            AX.X, ALU.max)
        mw_o = load("mw_out", (64, 256), BF16, pool=sc, tag="wenc")
        mb_o = load("mb_out", (128, 2), F32, pool=sc, tag="benc")
        map_mine = linear_fm(lambda ji, s0, s1: mfeat[0:64, s0:s1], 64, 256, MQ,
                             mw_o, 0, mb_o, 0, pool=sb, tag="map_mine")
        map_mine_f = sb.tile([128, 2 * MQ], F32, tag="map_mine_f")
        nc.vector.tensor_copy(map_mine_f[:, :], map_mine[:, :])

        # masks (one-time)
        m01_mm = build_mask("mm_qpos", "mm_kpos", MQ, M, "m01mm")
        m01_aa = build_mask("aa_qpos", "aa_kpos", AQ, A, "m01aa")
        m01_am = build_mask("aa_qpos", "mm_kpos", AQ, M, "m01am")

        if nlayers > 0:
            map_full = allgather(map_mine, MQ, tag="map_full")
            agent_full = allgather(agent_mine, AQ, tag="agent_full")
        map_mine_cur, map_mine_curf = map_mine, map_mine_f
        agent_mine_cur, agent_mine_curf = agent_mine, agent_mine_f

        for l in range(nlayers):
            Wmm = load(f"L{l}mm_W", (128, 6144), BF16, pool=wp, tag="Wblk")
            bmm = load(f"L{l}mm_b", (128, 18), F32, pool=wp, tag="bblk")
            lnmm = load(f"L{l}mm_ln", (2, 512), F32, pool=wp, tag="lnblk")
            map_mine_cur, map_mine_curf = block(Wmm, bmm, lnmm, map_mine_cur,
                                 map_mine_curf, MQ, map_full, M,
                                 m01_mm, out_tag="map_mine")
            map_full = allgather(map_mine_cur, MQ, tag="map_full")

            Waa = load(f"L{l}aa_W", (128, 6144), BF16, pool=wp, tag="Wblk")
            baa = load(f"L{l}aa_b", (128, 18), F32, pool=wp, tag="bblk")
            lnaa = load(f"L{l}aa_ln", (2, 512), F32, pool=wp, tag="lnblk")
            agent_mine_cur, agent_mine_curf = block(Waa, baa, lnaa, agent_mine_cur,
                                   agent_mine_curf, AQ, agent_full, A,
                                   m01_aa, out_tag="agent_mine")

            Wam = load(f"L{l}am_W", (128, 6144), BF16, pool=wp, tag="Wblk")
            bam = load(f"L{l}am_b", (128, 18), F32, pool=wp, tag="bblk")
            lnam = load(f"L{l}am_ln", (2, 512), F32, pool=wp, tag="lnblk")
            agent_mine_cur, agent_mine_curf = block(Wam, bam, lnam, agent_mine_cur,
                                   agent_mine_curf, AQ, map_full, M,
                                   m01_am, out_tag="agent_mine2")
            agent_full = allgather(agent_mine_cur, AQ, tag="agent_full")

        dma(out_map[:, :], map_mine_curf[:, :])
        dma(out_agent[:, :], agent_mine_curf[:, :])

    nc.finalize()
    return nc


_NC_CACHE = {}


def kernel(**inputs):
    global LAST_EXEC_TIME_NS
    nlayers = int(os.environ.get("KB_LAYERS", L))
    if nlayers not in _NC_CACHE:
        _NC_CACHE[nlayers] = build(nlayers)
    nc = _NC_CACHE[nlayers]
    in_maps = prepare_inputs(inputs)
    res = run_bass_kernel_spmd(nc, in_maps, core_ids=list(range(8)),
                               trace=bool(os.environ.get("KB_TRACE")))
    LAST_EXEC_TIME_NS = res.exec_time_ns
    map_out = np.zeros((B, M, D), np.float32)
    ag_out = np.zeros((B, A, D), np.float32)
    for c in range(8):
        s, h = c // 2, c % 2
        r = res.results[c]
        mo = r["out_map"].reshape(128, 2, MQ).transpose(1, 0, 2).reshape(D, MQ)
        map_out[s, h * MQ:(h + 1) * MQ] = mo.T
        ao = r["out_agent"].reshape(128, 2, AQ).transpose(1, 0, 2).reshape(D, AQ)
        ag_out[s, h * AQ:(h + 1) * AQ] = ao.T
    return (ag_out, map_out)
